# revision 13
# baseline (speedup 1.0000x reference)
"""Trainium2 Bass kernel for ExpertsChooseMaskedExpand MoE routing.

Math (reference):
    xd[b,e,c,i] = sum_t x[b,t,(e,i)] * dmask[b,t,e,c]            (dispatch)
    y[b,e,c,o]  = sum_i xd[b,e,c,i] * w[e,o,i] + bias[o]         (expert mm)
    out[b,t,o]  = sum_{e,c} y[b,e,c,o] * cmb[b,t,e,c]            (combine)

Restructured (combine applied before the weight matmul — 155 GF total
instead of 215 GF; the E expert matmuls fuse into one K=2048 matmul):
    xd[b,e][c,j] = sum_t dmask[b,e][t,c] * xr[b,e][t,j]
    zT[b,e][j,t] = sum_c xd[b,e][c,j] * cmbT[b,e][c,t]
    out[b][t,o]  = sum_{(e,j)} zT[b][(e,j),t] * wstack[(e,j),o] + s[b][t]*bias[o]
    where s[b][t] = sum_{e,c} cmb[b,t,e,c],  wstack[(e,j),o] = w[e,o,j]

Sharding: 8 cores = (batch b in 0..3) x (output half oh in 0..1). Each
core computes out[b][:, oh*4096:(oh+1)*4096] (returned o-major packed;
host unpacks) - no cross-core reduction. All matmuls run as float32r
(fp22, full PE rate).

Phase 3 runs transposed: stationary = weight block (j, o-tile), moving
= zT t-chunks, PSUM holds out^T (o, t). Each stationary is shared by
the two t-chunk matmuls. The s[t]*bias[o] rank-1 term is fused into
the PSUM->SBUF eviction on the vector engine.
"""

import numpy as np

B, T, E, C = 4, 1024, 4, 512
IN, OUT = 2048, 8192
P = 128
TT = T // P          # 8  t-tiles
CT = C // P          # 4  c-tiles per expert
JT = 4               # j-tiles per expert (i = 512)
EL = 2               # experts handled per core (expert-pair split)
KT = EL * JT         # 8 k-tiles for the fused matmul (K = 1024 per core)
OT = OUT // P        # 64 o-tiles of 128 (full output width per core)
TCH = 2              # t-chunks of 512

_CACHE = {}


def _build_nc():
    import concourse.mybir as mybir
    import concourse.tile as tile
    from concourse import bacc

    f32 = mybir.dt.float32
    f32r = mybir.dt.float32r

    nc = bacc.Bacc("TRN2", target_bir_lowering=False, debug=False, num_devices=8)
    x_t = nc.dram_tensor("x", (T, EL * 512), f32r, kind="ExternalInput")
    dm_t = nc.dram_tensor("dm", (T, EL, C), f32r, kind="ExternalInput")
    cT_t = nc.dram_tensor("cmbT", (EL, C, T), f32r, kind="ExternalInput")
    # wpk[p, ot, kt, oi] = wstack[h*1024 + kt*128+p, ot*128 + oi]
    wpk_t = nc.dram_tensor("wpk", (P, OT, KT, P), f32r, kind="ExternalInput")
    sb_t = nc.dram_tensor("sb", (P, T), f32, kind="ExternalInput")       # s bcast
    bT_t = nc.dram_tensor("biasT", (P, OT), f32, kind="ExternalInput")
    # out_pk[p, ot, tch, u] = out[tch*512+u, ot*128+p]
    o_t = nc.dram_tensor("out", (P, OT, TCH, 512), f32, kind="ExternalOutput")

    x_r = x_t.ap().rearrange("(tt p) f -> p tt f", p=P)        # [128, 8, 1024]
    dm_r = dm_t.ap().rearrange("(tt p) e c -> p tt e c", p=P)  # [128, 8, 2, 512]
    cT_r = cT_t.ap().rearrange("e (ct p) t -> p e ct t", p=P)  # [128, 2, 4, 1024]
    wpk_r = wpk_t.ap()                                         # [128, 64, 8, 128]
    o_r = o_t.ap()                                             # [128, 64, 2, 512]

    with tile.TileContext(nc) as tc:
        with (
            tc.tile_pool(name="persist", bufs=1) as persist,
            tc.tile_pool(name="wp", bufs=6) as wp,
            tc.tile_pool(name="op", bufs=3) as op,
        ):
            zT = persist.tile([P, KT, T], f32r)       # 64 KiB/partition
            sb_sb = persist.tile([P, T], f32)
            bT_sb = persist.tile([P, OT], f32)

            w_tiles = {}

            def load_w(ot):
                t = wp.tile([P, KT, P], f32r, tag="w", name=f"w_{ot}")
                nc.sync.dma_start(t, wpk_r[:, ot, :, :])
                w_tiles[ot] = t

            # ---- Phases 1+2: per-expert dispatch and combine ----
            with (
                tc.tile_pool(name="xdm", bufs=4) as xdm,
                tc.tile_pool(name="cp", bufs=3) as cp,
                tc.tile_pool(name="xdp", bufs=1) as xdp,
                tc.tile_pool(name="ps_a", bufs=4, space="PSUM") as ps_a,
                tc.tile_pool(name="ps_b", bufs=2, space="PSUM") as ps_b,
            ):
                for e in range(EL):
                    # phase 1: xd[c, j] = sum_t dm[t, c] * x[t, j]
                    # quarter-granularity loads; tt-outer so matmuls start early
                    xq, dmq = {}, {}
                    for qt in range(4):
                        qs = slice(qt * 2, qt * 2 + 2)
                        xq[qt] = xdm.tile(
                            [P, 2, 512], f32r, tag="x", name=f"x_{e}_{qt}"
                        )
                        dmq[qt] = xdm.tile(
                            [P, 2, 512], f32r, tag="dm", name=f"dm_{e}_{qt}"
                        )
                        nc.sync.dma_start(
                            xq[qt], x_r[:, qs, e * 512 : (e + 1) * 512]
                        )
                        nc.sync.dma_start(dmq[qt], dm_r[:, qs, e, :])
                    # combine loads for this expert, issued before phase-1 runs
                    c_ths = []
                    for th in range(2):
                        c_th = cp.tile([P, CT, 512], f32r, tag="c", name=f"c_{e}_{th}")
                        nc.sync.dma_start(
                            c_th, cT_r[:, e, :, th * 512 : (th + 1) * 512]
                        )
                        c_ths.append(c_th)
                    ps1 = [
                        ps_a.tile([P, 512], f32, tag="ps1", name=f"ps1_{e}_{ct}")
                        for ct in range(CT)
                    ]
                    for tt in range(TT):
                        qt, qi = tt // 2, tt % 2
                        for ct in range(CT):
                            nc.tensor.matmul(
                                ps1[ct],
                                dmq[qt][:, qi, ct * P : (ct + 1) * P],
                                xq[qt][:, qi, :],
                                start=(tt == 0),
                                stop=(tt == TT - 1),
                            )
                    xd_e = xdp.tile([P, CT, 512], f32r, tag="xd")
                    for ct in range(CT):
                        nc.vector.tensor_copy(xd_e[:, ct, :], ps1[ct])

                    # phase 2: zT[j, t] = sum_c xd[c, j] * cmbT[c, t]
                    for th in range(2):
                        c_th = c_ths[th]
                        for jt in range(JT):
                            ps2 = ps_b.tile([P, 512], f32, tag="ps2")
                            for ct in range(CT):
                                nc.tensor.matmul(
                                    ps2,
                                    xd_e[:, ct, jt * P : (jt + 1) * P],
                                    c_th[:, ct, :],
                                    start=(ct == 0),
                                    stop=(ct == CT - 1),
                                )
                            nc.vector.tensor_copy(
                                zT[:, e * JT + jt, th * 512 : (th + 1) * 512], ps2
                            )

                    # phase-3-only inputs, loaded after expert 0's stream
                    if e == 0:
                        nc.sync.dma_start(sb_sb, sb_t.ap())
                        nc.sync.dma_start(bT_sb, bT_t.ap())
                    # weight prefetch at the tail of each expert's DMA stream
                    for ot in {0: [0, 1], 1: [2, 3, 4, 5]}.get(e, []):
                        load_w(ot)

            # ---- Phase 3 (transposed): outT[o,t] = sum_kt w[kt].T @ zT[kt] ----
            with tc.tile_pool(name="ps_c", bufs=8, space="PSUM") as ps_c:
                for ot in range(OT):
                    if ot not in w_tiles:
                        load_w(ot)
                    if ot + 6 < OT and (ot + 6) not in w_tiles:
                        load_w(ot + 6)
                    psum = [
                        ps_c.tile([P, 512], f32, tag="ps3", name=f"ps3_{ot}_{i}")
                        for i in range(TCH)
                    ]
                    for kt in range(KT):
                        st = w_tiles[ot][:, kt, :]
                        for tch in range(TCH):
                            nc.tensor.matmul(
                                psum[tch],
                                st,
                                zT[:, kt, tch * 512 : (tch + 1) * 512],
                                start=(kt == 0),
                                stop=(kt == KT - 1),
                            )
                    for tch in range(TCH):
                        o_sb = op.tile([P, 512], f32, tag="o_sb")
                        # outT = s_bcast[:, tch] * biasT[:, ot] + psum
                        nc.vector.scalar_tensor_tensor(
                            o_sb,
                            sb_sb[:, tch * 512 : (tch + 1) * 512],
                            bT_sb[:, ot : ot + 1],
                            psum[tch],
                            mybir.AluOpType.mult,
                            mybir.AluOpType.add,
                        )
                        nc.sync.dma_start(o_r[:, ot, tch, :], o_sb)

    nc.compile()
    return nc


def _get_nc():
    if "nc" not in _CACHE:
        _CACHE["nc"] = _build_nc()
    return _CACHE["nc"]


def _prep_in_maps(x, combine_array, dispatch_mask, weight, bias):
    x = np.ascontiguousarray(x, dtype=np.float32)
    cmb = np.ascontiguousarray(combine_array, dtype=np.float32)
    dm = np.ascontiguousarray(dispatch_mask, dtype=np.float32)
    weight = np.ascontiguousarray(weight, dtype=np.float32)
    bias = np.ascontiguousarray(bias, dtype=np.float32)

    # combine transposed to (B, E, C, T) so that C lands on partitions
    cmbT = np.ascontiguousarray(cmb.transpose(0, 2, 3, 1))
    s = cmb.sum(axis=(2, 3))  # (B, T)
    sb = [np.ascontiguousarray(np.broadcast_to(s[b], (P, T))) for b in range(B)]
    # wstack[(e,j), o] = w[e, o, j];  w = weight.reshape(E, OUT, IN//E)
    w = weight.reshape(E, OUT, IN // E)
    wstack = np.ascontiguousarray(w.transpose(0, 2, 1)).reshape(IN, OUT)
    # expert-pair h owns wstack rows [h*1024, (h+1)*1024) over the full OUT
    wpk = []
    for h in range(2):
        wh = wstack[h * 1024 : (h + 1) * 1024, :].reshape(KT, P, OT, P)
        wpk.append(np.ascontiguousarray(wh.transpose(1, 2, 0, 3)))  # (p, ot, kt, oi)
    # bias applied once per pair: even cores get the real bias, odd get zeros
    bT = [
        np.ascontiguousarray(bias.reshape(OT, P).T),
        np.zeros((P, OT), dtype=np.float32),
    ]

    in_maps = []
    for k in range(8):
        b, h = k // 2, k % 2
        in_maps.append(
            {
                "x": np.ascontiguousarray(x[b][:, h * 1024 : (h + 1) * 1024]),
                "dm": np.ascontiguousarray(dm[b][:, 2 * h : 2 * h + 2, :]),
                "cmbT": np.ascontiguousarray(cmbT[b][2 * h : 2 * h + 2]),
                "wpk": wpk[h],
                "sb": sb[b],
                "biasT": bT[h],
            }
        )
    return in_maps


def run_spmd(in_maps, trace=False, **kwargs):
    from concourse.bass_utils import run_bass_kernel_spmd

    nc = _get_nc()
    return run_bass_kernel_spmd(
        nc, in_maps, core_ids=list(range(8)), trace=trace, **kwargs
    )


def kernel(x, combine_array, dispatch_mask, weight, bias, num_experts):
    assert int(num_experts) == E
    in_maps = _prep_in_maps(x, combine_array, dispatch_mask, weight, bias)
    res = run_spmd(in_maps)
    out = np.empty((B, T, OUT), dtype=np.float32)
    for b in range(B):
        pk = res.results[2 * b]["out"] + res.results[2 * b + 1]["out"]
        out[b] = pk.transpose(2, 3, 1, 0).reshape(T, OUT)  # (P,OT,TCH,512)->(t,o)
    return out


# revision 14
# speedup vs baseline: 1.0065x; 1.0065x over previous
"""Trainium2 Bass kernel for ExpertsChooseMaskedExpand MoE routing.

Math (reference):
    xd[b,e,c,i] = sum_t x[b,t,(e,i)] * dmask[b,t,e,c]            (dispatch)
    y[b,e,c,o]  = sum_i xd[b,e,c,i] * w[e,o,i] + bias[o]         (expert mm)
    out[b,t,o]  = sum_{e,c} y[b,e,c,o] * cmb[b,t,e,c]            (combine)

Restructured (combine applied before the weight matmul — 155 GF total
instead of 215 GF; the E expert matmuls fuse into one K=2048 matmul):
    xd[b,e][c,j] = sum_t dmask[b,e][t,c] * xr[b,e][t,j]
    zT[b,e][j,t] = sum_c xd[b,e][c,j] * cmbT[b,e][c,t]
    out[b][t,o]  = sum_{(e,j)} zT[b][(e,j),t] * wstack[(e,j),o] + s[b][t]*bias[o]
    where s[b][t] = sum_{e,c} cmb[b,t,e,c],  wstack[(e,j),o] = w[e,o,j]

Sharding: 8 cores = (batch b in 0..3) x (output half oh in 0..1). Each
core computes out[b][:, oh*4096:(oh+1)*4096] (returned o-major packed;
host unpacks) - no cross-core reduction. All matmuls run as float32r
(fp22, full PE rate).

Phase 3 runs transposed: stationary = weight block (j, o-tile), moving
= zT t-chunks, PSUM holds out^T (o, t). Each stationary is shared by
the two t-chunk matmuls. The s[t]*bias[o] rank-1 term is fused into
the PSUM->SBUF eviction on the vector engine.
"""

import numpy as np

B, T, E, C = 4, 1024, 4, 512
IN, OUT = 2048, 8192
P = 128
TT = T // P          # 8  t-tiles
CT = C // P          # 4  c-tiles per expert
JT = 4               # j-tiles per expert (i = 512)
EL = 2               # experts handled per core (expert-pair split)
KT = EL * JT         # 8 k-tiles for the fused matmul (K = 1024 per core)
OT = OUT // P        # 64 o-tiles of 128 (full output width per core)
TCH = 2              # t-chunks of 512

_CACHE = {}


def _build_nc():
    import concourse.mybir as mybir
    import concourse.tile as tile
    from concourse import bacc

    f32 = mybir.dt.float32
    f32r = mybir.dt.float32r

    nc = bacc.Bacc("TRN2", target_bir_lowering=False, debug=False, num_devices=8)
    x_t = nc.dram_tensor("x", (T, EL * 512), f32r, kind="ExternalInput")
    dm_t = nc.dram_tensor("dm", (T, EL, C), f32r, kind="ExternalInput")
    cT_t = nc.dram_tensor("cmbT", (EL, C, T), f32r, kind="ExternalInput")
    # wpk[p, ot, kt, oi] = wstack[h*1024 + kt*128+p, ot*128 + oi]
    wpk_t = nc.dram_tensor("wpk", (P, OT, KT, P), f32r, kind="ExternalInput")
    sb_t = nc.dram_tensor("sb", (P, T), f32, kind="ExternalInput")       # s bcast
    bT_t = nc.dram_tensor("biasT", (P, OT), f32, kind="ExternalInput")
    # out_pk[p, ot, tch, u] = out[tch*512+u, ot*128+p]
    o_t = nc.dram_tensor("out", (P, OT, TCH, 512), f32, kind="ExternalOutput")

    x_r = x_t.ap().rearrange("(tt p) f -> p tt f", p=P)        # [128, 8, 1024]
    dm_r = dm_t.ap().rearrange("(tt p) e c -> p tt e c", p=P)  # [128, 8, 2, 512]
    cT_r = cT_t.ap().rearrange("e (ct p) t -> p e ct t", p=P)  # [128, 2, 4, 1024]
    wpk_r = wpk_t.ap()                                         # [128, 64, 8, 128]
    o_r = o_t.ap()                                             # [128, 64, 2, 512]

    with tile.TileContext(nc) as tc:
        with (
            tc.tile_pool(name="persist", bufs=1) as persist,
            tc.tile_pool(name="wp", bufs=8) as wp,
            tc.tile_pool(name="op", bufs=3) as op,
        ):
            zT = persist.tile([P, KT, T], f32r)       # 64 KiB/partition
            sb_sb = persist.tile([P, T], f32)
            bT_sb = persist.tile([P, OT], f32)

            w_tiles = {}

            def load_w(ot):
                t = wp.tile([P, KT, P], f32r, tag="w", name=f"w_{ot}")
                nc.sync.dma_start(t, wpk_r[:, ot, :, :])
                w_tiles[ot] = t

            # ---- Phases 1+2: per-expert dispatch and combine ----
            with (
                tc.tile_pool(name="xdm", bufs=6) as xdm,
                tc.tile_pool(name="cp", bufs=4) as cp,
                tc.tile_pool(name="xdp", bufs=2) as xdp,
                tc.tile_pool(name="ps_a", bufs=4, space="PSUM") as ps_a,
                tc.tile_pool(name="ps_b", bufs=2, space="PSUM") as ps_b,
            ):
                for e in range(EL):
                    # phase 1: xd[c, j] = sum_t dm[t, c] * x[t, j]
                    # quarter-granularity loads; tt-outer so matmuls start early
                    xq, dmq = {}, {}
                    for qt in range(4):
                        qs = slice(qt * 2, qt * 2 + 2)
                        xq[qt] = xdm.tile(
                            [P, 2, 512], f32r, tag="x", name=f"x_{e}_{qt}"
                        )
                        dmq[qt] = xdm.tile(
                            [P, 2, 512], f32r, tag="dm", name=f"dm_{e}_{qt}"
                        )
                        nc.sync.dma_start(
                            xq[qt], x_r[:, qs, e * 512 : (e + 1) * 512]
                        )
                        nc.sync.dma_start(dmq[qt], dm_r[:, qs, e, :])
                    # combine loads for this expert, issued before phase-1 runs
                    c_ths = []
                    for th in range(2):
                        c_th = cp.tile([P, CT, 512], f32r, tag="c", name=f"c_{e}_{th}")
                        nc.sync.dma_start(
                            c_th, cT_r[:, e, :, th * 512 : (th + 1) * 512]
                        )
                        c_ths.append(c_th)
                    ps1 = [
                        ps_a.tile([P, 512], f32, tag="ps1", name=f"ps1_{e}_{ct}")
                        for ct in range(CT)
                    ]
                    for tt in range(TT):
                        qt, qi = tt // 2, tt % 2
                        for ct in range(CT):
                            nc.tensor.matmul(
                                ps1[ct],
                                dmq[qt][:, qi, ct * P : (ct + 1) * P],
                                xq[qt][:, qi, :],
                                start=(tt == 0),
                                stop=(tt == TT - 1),
                            )
                    xd_e = xdp.tile([P, CT, 512], f32r, tag="xd")
                    for ct in range(CT):
                        nc.vector.tensor_copy(xd_e[:, ct, :], ps1[ct])

                    # phase 2: zT[j, t] = sum_c xd[c, j] * cmbT[c, t]
                    for th in range(2):
                        c_th = c_ths[th]
                        for jt in range(JT):
                            ps2 = ps_b.tile([P, 512], f32, tag="ps2")
                            for ct in range(CT):
                                nc.tensor.matmul(
                                    ps2,
                                    xd_e[:, ct, jt * P : (jt + 1) * P],
                                    c_th[:, ct, :],
                                    start=(ct == 0),
                                    stop=(ct == CT - 1),
                                )
                            nc.vector.tensor_copy(
                                zT[:, e * JT + jt, th * 512 : (th + 1) * 512], ps2
                            )

                    # phase-3-only inputs, loaded after expert 0's stream
                    if e == 0:
                        nc.sync.dma_start(sb_sb, sb_t.ap())
                        nc.sync.dma_start(bT_sb, bT_t.ap())
                    # weight prefetch at the tail of each expert's DMA stream
                    for ot in {0: [0, 1, 2, 3], 1: [4, 5, 6, 7]}.get(e, []):
                        load_w(ot)

            # ---- Phase 3 (transposed): outT[o,t] = sum_kt w[kt].T @ zT[kt] ----
            with tc.tile_pool(name="ps_c", bufs=8, space="PSUM") as ps_c:
                for ot in range(OT):
                    if ot not in w_tiles:
                        load_w(ot)
                    if ot + 8 < OT and (ot + 8) not in w_tiles:
                        load_w(ot + 8)
                    psum = [
                        ps_c.tile([P, 512], f32, tag="ps3", name=f"ps3_{ot}_{i}")
                        for i in range(TCH)
                    ]
                    for kt in range(KT):
                        st = w_tiles[ot][:, kt, :]
                        for tch in range(TCH):
                            nc.tensor.matmul(
                                psum[tch],
                                st,
                                zT[:, kt, tch * 512 : (tch + 1) * 512],
                                start=(kt == 0),
                                stop=(kt == KT - 1),
                            )
                    for tch in range(TCH):
                        o_sb = op.tile([P, 512], f32, tag="o_sb")
                        # outT = s_bcast[:, tch] * biasT[:, ot] + psum
                        nc.vector.scalar_tensor_tensor(
                            o_sb,
                            sb_sb[:, tch * 512 : (tch + 1) * 512],
                            bT_sb[:, ot : ot + 1],
                            psum[tch],
                            mybir.AluOpType.mult,
                            mybir.AluOpType.add,
                        )
                        nc.sync.dma_start(o_r[:, ot, tch, :], o_sb)

    nc.compile()
    return nc


def _get_nc():
    if "nc" not in _CACHE:
        _CACHE["nc"] = _build_nc()
    return _CACHE["nc"]


def _prep_in_maps(x, combine_array, dispatch_mask, weight, bias):
    x = np.ascontiguousarray(x, dtype=np.float32)
    cmb = np.ascontiguousarray(combine_array, dtype=np.float32)
    dm = np.ascontiguousarray(dispatch_mask, dtype=np.float32)
    weight = np.ascontiguousarray(weight, dtype=np.float32)
    bias = np.ascontiguousarray(bias, dtype=np.float32)

    # combine transposed to (B, E, C, T) so that C lands on partitions
    cmbT = np.ascontiguousarray(cmb.transpose(0, 2, 3, 1))
    s = cmb.sum(axis=(2, 3))  # (B, T)
    sb = [np.ascontiguousarray(np.broadcast_to(s[b], (P, T))) for b in range(B)]
    # wstack[(e,j), o] = w[e, o, j];  w = weight.reshape(E, OUT, IN//E)
    w = weight.reshape(E, OUT, IN // E)
    wstack = np.ascontiguousarray(w.transpose(0, 2, 1)).reshape(IN, OUT)
    # expert-pair h owns wstack rows [h*1024, (h+1)*1024) over the full OUT
    wpk = []
    for h in range(2):
        wh = wstack[h * 1024 : (h + 1) * 1024, :].reshape(KT, P, OT, P)
        wpk.append(np.ascontiguousarray(wh.transpose(1, 2, 0, 3)))  # (p, ot, kt, oi)
    # bias applied once per pair: even cores get the real bias, odd get zeros
    bT = [
        np.ascontiguousarray(bias.reshape(OT, P).T),
        np.zeros((P, OT), dtype=np.float32),
    ]

    in_maps = []
    for k in range(8):
        b, h = k // 2, k % 2
        in_maps.append(
            {
                "x": np.ascontiguousarray(x[b][:, h * 1024 : (h + 1) * 1024]),
                "dm": np.ascontiguousarray(dm[b][:, 2 * h : 2 * h + 2, :]),
                "cmbT": np.ascontiguousarray(cmbT[b][2 * h : 2 * h + 2]),
                "wpk": wpk[h],
                "sb": sb[b],
                "biasT": bT[h],
            }
        )
    return in_maps


def run_spmd(in_maps, trace=False, **kwargs):
    from concourse.bass_utils import run_bass_kernel_spmd

    nc = _get_nc()
    return run_bass_kernel_spmd(
        nc, in_maps, core_ids=list(range(8)), trace=trace, **kwargs
    )


def kernel(x, combine_array, dispatch_mask, weight, bias, num_experts):
    assert int(num_experts) == E
    in_maps = _prep_in_maps(x, combine_array, dispatch_mask, weight, bias)
    res = run_spmd(in_maps)
    out = np.empty((B, T, OUT), dtype=np.float32)
    for b in range(B):
        pk = res.results[2 * b]["out"] + res.results[2 * b + 1]["out"]
        out[b] = pk.transpose(2, 3, 1, 0).reshape(T, OUT)  # (P,OT,TCH,512)->(t,o)
    return out
